# revision 5
# baseline (speedup 1.0000x reference)
"""Trainium2 Bass kernel for nn_BinarizedConv2d.

Math: activation[d, o] = sum_k weight_noise[d, o, k] * x[d, k]
      out[d, o]        = activation[d, o] > bias_noise[d, o]
with D=128 directions, O=256 out channels, K=2304 reduction length.

Sharding: D is split across 8 NeuronCores (16 directions per core) —
embarrassingly parallel, no collectives.

Per-core kernel: for each direction, 18 TensorE matmuls (K-tiles of 128)
accumulate activation[256] in PSUM: the direction's x K-tile [128, 1] is the
stationary operand (M=1), the weight K-tile [128, 256] (pre-transposed on
host) is the moving operand. The PSUM result is copied to SBUF and compared
is_gt against bias on VectorE, producing uint8 0/1 which is DMA'd out.
(Copy + compare rather than one PSUM-reading compare: the TensorTensor ISA
struct only fits one sync-wait, and a fused compare would need two.)

dtype trick: weight_noise and x are exactly 0/1, which fp8e4 represents
exactly; matmul accumulation is always fp32 in PSUM, and popcounts <= 2304
are exact in fp32, so results are bit-identical to the fp32 reference while
moving 4x fewer bytes from HBM (the memory-bound term).
"""

import numpy as np
import ml_dtypes

D = 128          # directions (ES population)
O = 256          # out channels
K = 2304         # flattened reduction length
T = 18           # K tiles of 128
P = 128          # partitions / K-tile size
NCORES = 8
DPC = D // NCORES  # directions per core

FP8 = ml_dtypes.float8_e4m3

_nc_cache = {}


def _emit(tc, res_ap, wT_ap, xT_ap, bias_ap):
    """Emit the per-core program into TileContext tc."""
    import concourse.mybir as mybir

    nc = tc.nc
    fp8 = mybir.dt.float8e4
    f32 = mybir.dt.float32
    u8 = mybir.dt.uint8

    with (
        tc.tile_pool(name="w", bufs=DPC) as wp,
        tc.tile_pool(name="small", bufs=1) as sp,
        tc.tile_pool(name="act", bufs=DPC) as ap_pool,
        tc.tile_pool(name="ps", bufs=8, space="PSUM") as pp,
    ):
        x_tile = sp.tile([P, DPC * T], fp8)
        nc.sync.dma_start(out=x_tile[:], in_=xT_ap)
        bias_tile = sp.tile([1, DPC * O], f32)
        nc.sync.dma_start(out=bias_tile[:], in_=bias_ap)
        # DVE-local copy of bias: every compare's inputs are then produced on
        # DVE, so each DVE instruction carries at most ONE sync wait (the TPB
        # 64B instruction structs have a single wait slot).
        bias2_tile = sp.tile([1, DPC * O], f32)
        nc.vector.tensor_copy(out=bias2_tile[:], in_=bias_tile[:])
        res_tile = sp.tile([1, DPC * O], u8)

        for d in range(DPC):
            w_tile = wp.tile([P, T * O], fp8)
            nc.sync.dma_start(out=w_tile[:], in_=wT_ap[d])
            ps = pp.tile([1, O], f32)
            for t in range(T):
                nc.tensor.matmul(
                    ps[:],
                    x_tile[:, d * T + t : d * T + t + 1],
                    w_tile[:, t * O : (t + 1) * O],
                    start=(t == 0),
                    stop=(t == T - 1),
                )
            sl = slice(d * O, (d + 1) * O)
            act_tile = ap_pool.tile([1, O], f32)
            nc.vector.tensor_copy(out=act_tile[:], in_=ps[:])
            nc.vector.tensor_tensor(
                out=res_tile[:, sl],
                in0=act_tile[:],
                in1=bias2_tile[:, sl],
                op=mybir.AluOpType.is_gt,
            )

        nc.sync.dma_start(out=res_ap, in_=res_tile[:])


def _build():
    """Build the per-core Bass program (same NEFF on all 8 cores)."""
    import concourse.bacc as bacc
    import concourse.mybir as mybir
    from concourse.tile import TileContext

    # Bacc (not raw Bass): its compile() runs move_matmul_waits_to_ldweights,
    # which splits 2-wait matmuls into LDW-wait + MM-wait (the 64B TPB
    # instruction structs have a single sync-wait slot).
    nc = bacc.Bacc("TRN2", debug=False)

    fp8 = mybir.dt.float8e4
    f32 = mybir.dt.float32
    u8 = mybir.dt.uint8

    # wT[d, p, t*O + o] = weight_noise[d0+d, o, t*128+p]  (pre-transposed host side)
    wT = nc.dram_tensor("wT", [DPC, P, T * O], fp8, kind="ExternalInput")
    # xT[p, d*T + t] = x[d0+d, t*128+p]
    xT = nc.dram_tensor("xT", [P, DPC * T], fp8, kind="ExternalInput")
    # bias[0, d*O + o] = bias_noise[d0+d, o]
    bias = nc.dram_tensor("bias", [1, DPC * O], f32, kind="ExternalInput")
    # res[0, d*O + o] = out[d0+d, o]
    res = nc.dram_tensor("res", [1, DPC * O], u8, kind="ExternalOutput")

    with TileContext(nc) as tc:
        _emit(tc, res.ap(), wT.ap(), xT.ap(), bias.ap())
    nc.compile()
    return nc


def prepare_inputs(weight_noise, bias_noise, x):
    """Host-side dtype cast + layout transform + sharding. Exact (0/1 -> fp8)."""
    w8 = np.asarray(weight_noise).astype(FP8)           # [D, O, K]
    # wT[d, p, t, o] = w[d, o, t*128+p]
    wT = np.ascontiguousarray(
        w8.reshape(D, O, T, P).transpose(0, 3, 2, 1)
    ).reshape(D, P, T * O)
    x8 = np.asarray(x).astype(FP8)                      # [D, K]
    xTfull = np.ascontiguousarray(x8.reshape(D, T, P).transpose(2, 0, 1))  # [P, D, T]
    b32 = np.asarray(bias_noise).astype(np.float32)

    in_maps = []
    for c in range(NCORES):
        sl = slice(c * DPC, (c + 1) * DPC)
        in_maps.append(
            {
                "wT": np.ascontiguousarray(wT[sl]),
                "xT": np.ascontiguousarray(xTfull[:, sl, :]).reshape(P, DPC * T),
                "bias": np.ascontiguousarray(b32[sl]).reshape(1, DPC * O),
            }
        )
    return in_maps


def run(weight_noise, bias_noise, x, trace=False, **spmd_kwargs):
    """Run on the 8 NeuronCores; returns (bool [D, O] output, BassKernelResults)."""
    from concourse.bass_utils import run_bass_kernel_spmd

    in_maps = prepare_inputs(weight_noise, bias_noise, x)
    if "nc" in _nc_cache:
        nc = _nc_cache["nc"]
    else:
        nc = _nc_cache["nc"] = _build()
    r = run_bass_kernel_spmd(
        nc, in_maps, core_ids=list(range(NCORES)), trace=trace, **spmd_kwargs
    )
    out = np.concatenate(
        [r.results[c]["res"].reshape(DPC, O) for c in range(NCORES)], axis=0
    )
    return out.astype(bool), r


def kernel(weight_noise, bias_noise, x):
    out, _ = run(weight_noise, bias_noise, x)
    return out


# revision 6
# speedup vs baseline: 1.0183x; 1.0183x over previous
"""Trainium2 Bass kernel for nn_BinarizedConv2d.

Math: activation[d, o] = sum_k weight_noise[d, o, k] * x[d, k]
      out[d, o]        = activation[d, o] > bias_noise[d, o]
with D=128 directions, O=256 out channels, K=2304 reduction length.

Sharding: D is split across 8 NeuronCores (16 directions per core) —
embarrassingly parallel, no collectives.

Per-core kernel: for each direction, 18 TensorE matmuls (K-tiles of 128)
accumulate activation[256] in PSUM: the direction's x K-tile [128, 1] is the
stationary operand (M=1), the weight K-tile [128, 256] (pre-transposed on
host) is the moving operand. The PSUM result is copied to SBUF and compared
is_gt against bias on VectorE, producing uint8 0/1 which is DMA'd out.
(Copy + compare rather than one PSUM-reading compare: the TensorTensor ISA
struct only fits one sync-wait, and a fused compare would need two.)

dtype trick: weight_noise and x are exactly 0/1, which fp8e4 represents
exactly; matmul accumulation is always fp32 in PSUM, and popcounts <= 2304
are exact in fp32, so results are bit-identical to the fp32 reference while
moving 4x fewer bytes from HBM (the memory-bound term).
"""

import numpy as np
import ml_dtypes

D = 128          # directions (ES population)
O = 256          # out channels
K = 2304         # flattened reduction length
T = 18           # K tiles of 128
P = 128          # partitions / K-tile size
NCORES = 8
DPC = D // NCORES  # directions per core

FP8 = ml_dtypes.float8_e4m3

_nc_cache = {}


def _emit(tc, res_ap, wT_ap, xT_ap, bias_ap):
    """Emit the per-core program into TileContext tc."""
    import concourse.mybir as mybir

    nc = tc.nc
    fp8 = mybir.dt.float8e4
    f32 = mybir.dt.float32
    u8 = mybir.dt.uint8

    NQ = DPC // 4  # quads of 4 directions, col-tiled across the PE array

    with (
        tc.tile_pool(name="w", bufs=DPC) as wp,
        tc.tile_pool(name="small", bufs=1) as sp,
        tc.tile_pool(name="act", bufs=DPC) as ap_pool,
        tc.tile_pool(name="ps", bufs=4, space="PSUM") as pp,
    ):
        x_tile = sp.tile([P, DPC * T], fp8)
        nc.sync.dma_start(out=x_tile[:], in_=xT_ap)
        bias_tile = sp.tile([1, DPC * O], f32)
        nc.sync.dma_start(out=bias_tile[:], in_=bias_ap)
        # DVE-local copy of bias: every compare's inputs are then produced on
        # DVE, so each DVE instruction carries at most ONE sync wait (the TPB
        # 64B instruction structs have a single wait slot).
        bias2_tile = sp.tile([1, DPC * O], f32)
        nc.vector.tensor_copy(out=bias2_tile[:], in_=bias_tile[:])
        res_tile = sp.tile([1, DPC * O], u8)

        w_tiles = []
        for d in range(DPC):
            w_tile = wp.tile([P, T * O], fp8)
            nc.sync.dma_start(out=w_tile[:], in_=wT_ap[d])
            w_tiles.append(w_tile)

        for q in range(NQ):
            # One PSUM bank per quad; direction j of the quad accumulates in
            # partition row 32*j via PE column-group tiling, so the 4 matvecs
            # run concurrently in the array (independent 32-col groups).
            ps = pp.tile([P, O], f32)
            for t in range(T):
                for j in range(4):
                    d = q * 4 + j
                    nc.tensor.matmul(
                        ps[32 * j : 32 * j + 1, :],
                        x_tile[:, d * T + t : d * T + t + 1],
                        w_tiles[d][:, t * O : (t + 1) * O],
                        start=(t == 0),
                        stop=(t == T - 1),
                        tile_position=(0, 32 * j),
                    )
            for j in range(4):
                d = q * 4 + j
                sl = slice(d * O, (d + 1) * O)
                act_tile = ap_pool.tile([1, O], f32)
                nc.vector.tensor_copy(out=act_tile[:], in_=ps[32 * j : 32 * j + 1, :])
                nc.vector.tensor_tensor(
                    out=res_tile[:, sl],
                    in0=act_tile[:],
                    in1=bias2_tile[:, sl],
                    op=mybir.AluOpType.is_gt,
                )

        nc.sync.dma_start(out=res_ap, in_=res_tile[:])


def _build():
    """Build the per-core Bass program (same NEFF on all 8 cores)."""
    import concourse.bacc as bacc
    import concourse.mybir as mybir
    from concourse.tile import TileContext

    # Bacc (not raw Bass): its compile() runs move_matmul_waits_to_ldweights,
    # which splits 2-wait matmuls into LDW-wait + MM-wait (the 64B TPB
    # instruction structs have a single sync-wait slot).
    nc = bacc.Bacc("TRN2", debug=False)

    fp8 = mybir.dt.float8e4
    f32 = mybir.dt.float32
    u8 = mybir.dt.uint8

    # wT[d, p, t*O + o] = weight_noise[d0+d, o, t*128+p]  (pre-transposed host side)
    wT = nc.dram_tensor("wT", [DPC, P, T * O], fp8, kind="ExternalInput")
    # xT[p, d*T + t] = x[d0+d, t*128+p]
    xT = nc.dram_tensor("xT", [P, DPC * T], fp8, kind="ExternalInput")
    # bias[0, d*O + o] = bias_noise[d0+d, o]
    bias = nc.dram_tensor("bias", [1, DPC * O], f32, kind="ExternalInput")
    # res[0, d*O + o] = out[d0+d, o]
    res = nc.dram_tensor("res", [1, DPC * O], u8, kind="ExternalOutput")

    with TileContext(nc) as tc:
        _emit(tc, res.ap(), wT.ap(), xT.ap(), bias.ap())
    nc.compile()
    return nc


def prepare_inputs(weight_noise, bias_noise, x):
    """Host-side dtype cast + layout transform + sharding. Exact (0/1 -> fp8)."""
    w8 = np.asarray(weight_noise).astype(FP8)           # [D, O, K]
    # wT[d, p, t, o] = w[d, o, t*128+p]
    wT = np.ascontiguousarray(
        w8.reshape(D, O, T, P).transpose(0, 3, 2, 1)
    ).reshape(D, P, T * O)
    x8 = np.asarray(x).astype(FP8)                      # [D, K]
    xTfull = np.ascontiguousarray(x8.reshape(D, T, P).transpose(2, 0, 1))  # [P, D, T]
    b32 = np.asarray(bias_noise).astype(np.float32)

    in_maps = []
    for c in range(NCORES):
        sl = slice(c * DPC, (c + 1) * DPC)
        in_maps.append(
            {
                "wT": np.ascontiguousarray(wT[sl]),
                "xT": np.ascontiguousarray(xTfull[:, sl, :]).reshape(P, DPC * T),
                "bias": np.ascontiguousarray(b32[sl]).reshape(1, DPC * O),
            }
        )
    return in_maps


def run(weight_noise, bias_noise, x, trace=False, **spmd_kwargs):
    """Run on the 8 NeuronCores; returns (bool [D, O] output, BassKernelResults)."""
    from concourse.bass_utils import run_bass_kernel_spmd

    in_maps = prepare_inputs(weight_noise, bias_noise, x)
    if "nc" in _nc_cache:
        nc = _nc_cache["nc"]
    else:
        nc = _nc_cache["nc"] = _build()
    r = run_bass_kernel_spmd(
        nc, in_maps, core_ids=list(range(NCORES)), trace=trace, **spmd_kwargs
    )
    out = np.concatenate(
        [r.results[c]["res"].reshape(DPC, O) for c in range(NCORES)], axis=0
    )
    return out.astype(bool), r


def kernel(weight_noise, bias_noise, x):
    out, _ = run(weight_noise, bias_noise, x)
    return out


# revision 8
# speedup vs baseline: 1.0468x; 1.0280x over previous
"""Trainium2 Bass kernel for nn_BinarizedConv2d.

Math: activation[d, o] = sum_k weight_noise[d, o, k] * x[d, k]
      out[d, o]        = activation[d, o] > bias_noise[d, o]
with D=128 directions, O=256 out channels, K=2304 reduction length.

Sharding: D is split across 8 NeuronCores (16 directions per core) —
embarrassingly parallel, no collectives.

Per-core kernel: for each direction, 18 TensorE matmuls (K-tiles of 128)
accumulate activation[256] in PSUM: the direction's x K-tile [128, 1] is the
stationary operand (M=1), the weight K-tile [128, 256] (pre-transposed on
host) is the moving operand. The PSUM result is copied to SBUF and compared
is_gt against bias on VectorE, producing uint8 0/1 which is DMA'd out.
(Copy + compare rather than one PSUM-reading compare: the TensorTensor ISA
struct only fits one sync-wait, and a fused compare would need two.)

dtype trick: weight_noise and x are exactly 0/1, which fp8e4 represents
exactly; matmul accumulation is always fp32 in PSUM, and popcounts <= 2304
are exact in fp32, so results are bit-identical to the fp32 reference while
moving 4x fewer bytes from HBM (the memory-bound term).
"""

import numpy as np
import ml_dtypes

D = 128          # directions (ES population)
O = 256          # out channels
K = 2304         # flattened reduction length
T = 18           # K tiles of 128
P = 128          # partitions / K-tile size
NCORES = 8
DPC = D // NCORES  # directions per core

FP8 = ml_dtypes.float8_e4m3

_nc_cache = {}


def _emit(tc, res_ap, wT_ap, xT_ap, bias_ap):
    """Emit the per-core program into TileContext tc."""
    import concourse.mybir as mybir

    nc = tc.nc
    fp8 = mybir.dt.float8e4
    f32 = mybir.dt.float32
    u8 = mybir.dt.uint8

    NQ = DPC // 4  # quads of 4 directions, col-tiled across the PE array
    TH = T // 2    # k-tiles per half (W DMA'd in halves for pipelining)

    with (
        tc.tile_pool(name="w", bufs=1) as wp,
        tc.tile_pool(name="small", bufs=1) as sp,
        tc.tile_pool(name="act", bufs=DPC) as ap_pool,
        tc.tile_pool(name="ps", bufs=4, space="PSUM") as pp,
    ):
        x_tile = sp.tile([P, DPC * T], fp8)
        nc.scalar.dma_start(out=x_tile[:], in_=xT_ap)
        bias_tile = sp.tile([1, DPC * O], f32)
        nc.scalar.dma_start(out=bias_tile[:], in_=bias_ap)
        # DVE-local copy of bias: every compare's inputs are then produced on
        # DVE, so each DVE instruction carries at most ONE sync wait (the TPB
        # 64B instruction structs have a single wait slot).
        bias2_tile = sp.tile([1, DPC * O], f32)
        nc.vector.tensor_copy(out=bias2_tile[:], in_=bias_tile[:])
        res_tile = sp.tile([1, DPC * O], u8)

        # Per-direction W in two half tiles (k-tiles 0..8 / 9..17), issued in
        # consume order and alternating between the two HWDGE rings (SP + ACT)
        # so both descriptor queues stream concurrently.
        w_half = [[None] * 2 for _ in range(DPC)]
        issue = 0
        for q in range(NQ):
            for h in range(2):
                for j in range(4):
                    d = q * 4 + j
                    wt = wp.tile([P, TH * O], fp8, tag=f"w{d}h{h}")
                    eng = nc.sync if issue % 2 == 0 else nc.scalar
                    eng.dma_start(
                        out=wt[:], in_=wT_ap[d][:, h * TH * O : (h + 1) * TH * O]
                    )
                    w_half[d][h] = wt
                    issue += 1

        for q in range(NQ):
            # One PSUM bank per quad; direction j of the quad accumulates in
            # partition row 32*j via PE column-group tiling, so the 4 matvecs
            # run concurrently in the array (independent 32-col groups).
            ps = pp.tile([P, O], f32)
            for t in range(T):
                h, th = divmod(t, TH)
                for j in range(4):
                    d = q * 4 + j
                    nc.tensor.matmul(
                        ps[32 * j : 32 * j + 1, :],
                        x_tile[:, d * T + t : d * T + t + 1],
                        w_half[d][h][:, th * O : (th + 1) * O],
                        start=(t == 0),
                        stop=(t == T - 1),
                        tile_position=(0, 32 * j),
                    )
            for j in range(4):
                d = q * 4 + j
                sl = slice(d * O, (d + 1) * O)
                act_tile = ap_pool.tile([1, O], f32)
                nc.vector.tensor_copy(out=act_tile[:], in_=ps[32 * j : 32 * j + 1, :])
                nc.vector.tensor_tensor(
                    out=res_tile[:, sl],
                    in0=act_tile[:],
                    in1=bias2_tile[:, sl],
                    op=mybir.AluOpType.is_gt,
                )

        nc.sync.dma_start(out=res_ap, in_=res_tile[:])


def _build():
    """Build the per-core Bass program (same NEFF on all 8 cores)."""
    import concourse.bacc as bacc
    import concourse.mybir as mybir
    from concourse.tile import TileContext

    # Bacc (not raw Bass): its compile() runs move_matmul_waits_to_ldweights,
    # which splits 2-wait matmuls into LDW-wait + MM-wait (the 64B TPB
    # instruction structs have a single sync-wait slot).
    nc = bacc.Bacc("TRN2", debug=False)

    fp8 = mybir.dt.float8e4
    f32 = mybir.dt.float32
    u8 = mybir.dt.uint8

    # wT[d, p, t*O + o] = weight_noise[d0+d, o, t*128+p]  (pre-transposed host side)
    wT = nc.dram_tensor("wT", [DPC, P, T * O], fp8, kind="ExternalInput")
    # xT[p, d*T + t] = x[d0+d, t*128+p]
    xT = nc.dram_tensor("xT", [P, DPC * T], fp8, kind="ExternalInput")
    # bias[0, d*O + o] = bias_noise[d0+d, o]
    bias = nc.dram_tensor("bias", [1, DPC * O], f32, kind="ExternalInput")
    # res[0, d*O + o] = out[d0+d, o]
    res = nc.dram_tensor("res", [1, DPC * O], u8, kind="ExternalOutput")

    with TileContext(nc) as tc:
        _emit(tc, res.ap(), wT.ap(), xT.ap(), bias.ap())
    nc.compile()
    return nc


def prepare_inputs(weight_noise, bias_noise, x):
    """Host-side dtype cast + layout transform + sharding. Exact (0/1 -> fp8)."""
    w8 = np.asarray(weight_noise).astype(FP8)           # [D, O, K]
    # wT[d, p, t, o] = w[d, o, t*128+p]
    wT = np.ascontiguousarray(
        w8.reshape(D, O, T, P).transpose(0, 3, 2, 1)
    ).reshape(D, P, T * O)
    x8 = np.asarray(x).astype(FP8)                      # [D, K]
    xTfull = np.ascontiguousarray(x8.reshape(D, T, P).transpose(2, 0, 1))  # [P, D, T]
    b32 = np.asarray(bias_noise).astype(np.float32)

    in_maps = []
    for c in range(NCORES):
        sl = slice(c * DPC, (c + 1) * DPC)
        in_maps.append(
            {
                "wT": np.ascontiguousarray(wT[sl]),
                "xT": np.ascontiguousarray(xTfull[:, sl, :]).reshape(P, DPC * T),
                "bias": np.ascontiguousarray(b32[sl]).reshape(1, DPC * O),
            }
        )
    return in_maps


def run(weight_noise, bias_noise, x, trace=False, **spmd_kwargs):
    """Run on the 8 NeuronCores; returns (bool [D, O] output, BassKernelResults)."""
    from concourse.bass_utils import run_bass_kernel_spmd

    in_maps = prepare_inputs(weight_noise, bias_noise, x)
    if "nc" in _nc_cache:
        nc = _nc_cache["nc"]
    else:
        nc = _nc_cache["nc"] = _build()
    r = run_bass_kernel_spmd(
        nc, in_maps, core_ids=list(range(NCORES)), trace=trace, **spmd_kwargs
    )
    out = np.concatenate(
        [r.results[c]["res"].reshape(DPC, O) for c in range(NCORES)], axis=0
    )
    return out.astype(bool), r


def kernel(weight_noise, bias_noise, x):
    out, _ = run(weight_noise, bias_noise, x)
    return out


# revision 31
# speedup vs baseline: 1.0719x; 1.0239x over previous
"""Trainium2 Bass kernel for nn_BinarizedConv2d.

Math: activation[d, o] = sum_k weight_noise[d, o, k] * x[d, k]
      out[d, o]        = activation[d, o] > bias_noise[d, o]
with D=128 directions, O=256 out channels, K=2304 reduction length.

Sharding: D is split across 8 NeuronCores (16 directions per core) —
embarrassingly parallel, no collectives.

Per-core kernel: for each direction, 18 TensorE matmuls (K-tiles of 128)
accumulate activation[256] in PSUM: the direction's x K-tile [128, 1] is the
stationary operand (M=1), the weight K-tile [128, 256] (pre-transposed on
host) is the moving operand. The PSUM result is copied to SBUF and compared
is_gt against bias on VectorE, producing uint8 0/1 which is DMA'd out.
(Copy + compare rather than one PSUM-reading compare: the TensorTensor ISA
struct only fits one sync-wait, and a fused compare would need two.)

dtype trick: weight_noise and x are exactly 0/1, which fp8e4 represents
exactly; matmul accumulation is always fp32 in PSUM, and popcounts <= 2304
are exact in fp32, so results are bit-identical to the fp32 reference while
moving 4x fewer bytes from HBM (the memory-bound term).
"""

import numpy as np
import ml_dtypes

D = 128          # directions (ES population)
O = 256          # out channels
K = 2304         # flattened reduction length
T = 18           # K tiles of 128
P = 128          # partitions / K-tile size
NCORES = 8
DPC = D // NCORES  # directions per core

FP8 = ml_dtypes.float8_e4m3

_nc_cache = {}


def _emit(tc, res_ap, wT_ap, xT_ap, bias_ap):
    """Emit the per-core program into TileContext tc."""
    import concourse.mybir as mybir

    nc = tc.nc
    fp8 = mybir.dt.float8e4
    f32 = mybir.dt.float32
    u8 = mybir.dt.uint8

    NQ = DPC // 4  # quads of 4 directions, col-tiled across the PE array
    TH = T // 2    # k-tiles per half (W DMA'd in halves for pipelining)

    with (
        tc.tile_pool(name="w", bufs=1) as wp,
        tc.tile_pool(name="small", bufs=1) as sp,
        tc.tile_pool(name="act", bufs=1) as ap_pool,
        tc.tile_pool(name="ps", bufs=1, space="PSUM") as pp,
    ):
        # x first on the SP ring so no W chunk queues ahead of it (every
        # matmul depends on x).
        x_tile = sp.tile([P, DPC * T], fp8)
        nc.sync.dma_start(out=x_tile[:], in_=xT_ap)
        # bias arrives as 4 DRAM rows (row j = directions 4q+j over quads q),
        # each DMA'd with a 0-step partition AP so partition 32j+r holds
        # direction 4q+j's bias for all r: the per-quad compare is one
        # full-width DVE op.
        bias_rep = sp.tile([P, NQ * O], f32)
        for j in range(4):
            nc.scalar.dma_start(
                out=bias_rep[32 * j : 32 * (j + 1), :],
                in_=bias_ap[j : j + 1, :].broadcast_to((32, NQ * O)),
            )
        # DVE probe depending on the broadcasts: later DVE compares then carry
        # no GpSimd wait (the TPB 64B instruction structs have a single
        # sync-wait slot, and the compares already need a DVE-self wait).
        probe_tile = sp.tile([1, 4], f32)
        nc.vector.tensor_copy(out=probe_tile[:], in_=bias_rep[0:1, 0:4])
        res_tile = sp.tile([P, NQ * O], u8)

        # W arrives as 8 chunks of [P, TH*4*O] (one per quad-half, 1.18 MB,
        # contiguous per partition for big SDMA descriptors, th-major so
        # k-tile ranges are contiguous), issued in consume order and
        # alternating between the two HWDGE rings (SP + ACT) so both
        # descriptor queues stream concurrently. The final chunk is split
        # into 3 pieces so the PE only trails the stream end by ~3 k-tiles.
        NPIECE = 3
        PTH = TH // NPIECE
        w_chunk = []
        for i in range(2 * NQ):
            if i < 2 * NQ - 1:
                wt = wp.tile([P, TH * 4 * O], fp8, tag=f"wc{i}")
                eng = nc.sync if i % 2 == 0 else nc.scalar
                eng.dma_start(out=wt[:], in_=wT_ap[i])
                w_chunk.append(wt)
            else:
                pieces = []
                for pz in range(NPIECE):
                    wt = wp.tile([P, PTH * 4 * O], fp8, tag=f"wc{i}p{pz}")
                    eng = nc.sync if pz % 2 == 0 else nc.scalar
                    eng.dma_start(
                        out=wt[:],
                        in_=wT_ap[i][:, pz * PTH * 4 * O : (pz + 1) * PTH * 4 * O],
                    )
                    pieces.append(wt)
                w_chunk.append(pieces)

        # One PSUM tile spanning all 8 banks; quad q accumulates in bank q's
        # first 256 columns. Direction j of a quad accumulates in partition
        # rows 32j..32j+31 via PE column-group tiling, so the 4 matvecs run
        # concurrently in the array (independent 32-col groups) and the quad
        # epilogue is full-width on DVE. skip_group_check: the per-(q,j)
        # accumulation groups are disjoint (partition x bank), but the group
        # tracker models PSUM flat and can't represent partition-ranged
        # groups; actual has_written accumulate semantics are per element.
        ps_all = pp.tile([P, 8 * 2 * O], f32)
        for q in range(NQ):
            win = slice(q * 2 * O, q * 2 * O + O)
            for t in range(T):
                h, th = divmod(t, TH)
                src = w_chunk[q * 2 + h]
                if isinstance(src, list):
                    src = src[th // PTH]
                    th = th % PTH
                for j in range(4):
                    d = q * 4 + j
                    # lhsT is x broadcast over 32 columns (step-0 AP): all 32
                    # rows of PE column-group j compute the same matvec, so
                    # the activation fills partitions 32j..32j+31.
                    nc.tensor.matmul(
                        ps_all[32 * j : 32 * (j + 1), win],
                        x_tile[:, d * T + t : d * T + t + 1].broadcast_to((P, 32)),
                        src[:, (th * 4 + j) * O : (th * 4 + j + 1) * O],
                        start=(t == 0),
                        stop=(t == T - 1),
                        tile_position=(0, 32 * j),
                        skip_group_check=True,
                    )
            sl = slice(q * O, (q + 1) * O)
            act_tile = ap_pool.tile([P, O], f32, tag=f"act{q}")
            nc.vector.tensor_copy(out=act_tile[:], in_=ps_all[:, win])
            nc.vector.tensor_tensor(
                out=res_tile[:, sl],
                in0=act_tile[:],
                in1=bias_rep[:, sl],
                op=mybir.AluOpType.is_gt,
            )
            # Per-quad result store: quads 0..2 fly out while later quads
            # still compute; only quad 3's small store is on the tail.
            nc.sync.dma_start(out=res_ap[:, sl], in_=res_tile[0:P:32, sl])


def _build():
    """Build the per-core Bass program (same NEFF on all 8 cores)."""
    import concourse.bacc as bacc
    import concourse.mybir as mybir
    from concourse.tile import TileContext

    # Bacc (not raw Bass): its compile() runs move_matmul_waits_to_ldweights,
    # which splits 2-wait matmuls into LDW-wait + MM-wait (the 64B TPB
    # instruction structs have a single sync-wait slot).
    nc = bacc.Bacc("TRN2", debug=False, enable_asserts=False)

    fp8 = mybir.dt.float8e4
    f32 = mybir.dt.float32
    u8 = mybir.dt.uint8

    # wT[q*2+h, p, (j*9+th)*O + o] = weight_noise[d0+4q+j, o, (h*9+th)*128+p]
    # (pre-transposed host side; 8 chunks of one quad-half each)
    wT = nc.dram_tensor("wT", [DPC // 2, P, (T // 2) * 4 * O], fp8, kind="ExternalInput")
    # xT[p, d*T + t] = x[d0+d, t*128+p]
    xT = nc.dram_tensor("xT", [P, DPC * T], fp8, kind="ExternalInput")
    # bias[j, q*O + o] = bias_noise[d0+4q+j, o]
    bias = nc.dram_tensor("bias", [4, (DPC // 4) * O], f32, kind="ExternalInput")
    # res[j, q*O + o] = out[d0+4q+j, o]
    res = nc.dram_tensor("res", [4, (DPC // 4) * O], u8, kind="ExternalOutput")

    with TileContext(nc) as tc:
        _emit(tc, res.ap(), wT.ap(), xT.ap(), bias.ap())
    nc.compile()
    return nc


def prepare_inputs(weight_noise, bias_noise, x):
    """Host-side dtype cast + layout transform + sharding. Exact (0/1 -> fp8)."""
    w8 = np.asarray(weight_noise).astype(FP8)           # [D, O, K]
    # wT[d, p, t, o] = w[d, o, t*128+p]
    wT = np.ascontiguousarray(
        w8.reshape(D, O, T, P).transpose(0, 3, 2, 1)
    ).reshape(D, P, T * O)
    x8 = np.asarray(x).astype(FP8)                      # [D, K]
    xTfull = np.ascontiguousarray(x8.reshape(D, T, P).transpose(2, 0, 1))  # [P, D, T]
    b32 = np.asarray(bias_noise).astype(np.float32)

    in_maps = []
    for c in range(NCORES):
        sl = slice(c * DPC, (c + 1) * DPC)
        # [d, p, t, o] -> [q, h, p, th, j, o] -> 8 chunks of one quad-half
        wc = (
            wT[sl]
            .reshape(DPC // 4, 4, P, 2, T // 2, O)
            .transpose(0, 3, 2, 4, 1, 5)
            .reshape(DPC // 2, P, (T // 2) * 4 * O)
        )
        # bias[j, q*O+o] = bias_noise[d0 + 4q + j, o]
        bc = (
            b32[sl]
            .reshape(DPC // 4, 4, O)
            .transpose(1, 0, 2)
            .reshape(4, (DPC // 4) * O)
        )
        in_maps.append(
            {
                "wT": np.ascontiguousarray(wc),
                "xT": np.ascontiguousarray(xTfull[:, sl, :]).reshape(P, DPC * T),
                "bias": np.ascontiguousarray(bc),
            }
        )
    return in_maps


def run(weight_noise, bias_noise, x, trace=False, **spmd_kwargs):
    """Run on the 8 NeuronCores; returns (bool [D, O] output, BassKernelResults)."""
    from concourse.bass_utils import run_bass_kernel_spmd

    in_maps = prepare_inputs(weight_noise, bias_noise, x)
    if "nc" in _nc_cache:
        nc = _nc_cache["nc"]
    else:
        nc = _nc_cache["nc"] = _build()
    r = run_bass_kernel_spmd(
        nc, in_maps, core_ids=list(range(NCORES)), trace=trace, **spmd_kwargs
    )
    out = np.concatenate(
        [
            r.results[c]["res"]
            .reshape(4, DPC // 4, O)
            .transpose(1, 0, 2)
            .reshape(DPC, O)
            for c in range(NCORES)
        ],
        axis=0,
    )
    return out.astype(bool), r


def kernel(weight_noise, bias_noise, x):
    out, _ = run(weight_noise, bias_noise, x)
    return out
